# revision 60
# baseline (speedup 1.0000x reference)
"""Trainium2 Bass kernel for nn_DomainGCN (4-layer GCN + MLP head), 8 NeuronCores.

Strategy (graph/data parallel, per sharding hint):
  - Nodes sharded contiguously: core r owns rows [r*6272, (r+1)*6272) (padded).
  - Reformulation: Agg(h@W) with per-edge weight dis[src]*dis[dst] and
    self-loops as a dedicated per-block self tile (weight dis[j]^2):
      z = h @ W            (dense, f32r matmuls, feature-major h in SBUF)
      z -> zdt -> AllGather (every core gets the full z, node-major, in DRAM)
      agg = S.T @ z[src]   (dma_gather of edge messages + one-hot-weighted
                            matmuls accumulating per 128-dst-node block in PSUM)
      h' = relu(agg + b)   (DVE add + ACT relu), PE-transposed back to
                            feature-major for the next dense matmul.
  - v2: SOFTWARE-PIPELINED emission: agg(l-1, b) and dense(l, b) are emitted
    interleaved per block, so the in-order engines overlap the aggregation of
    layer l-1 with the dense matmuls of layer l, and the AllGather halves of
    layer l fire mid-aggregation (hidden behind gather/matmul work).
    Per-layer z buffers keep this race-free across layers.
  - z payload dtype: fp8e4m3 for layers 1-2 (halves the dominant gather/
    AllGather traffic; verified rel-err ~8.5e-3 vs the 2e-2 gate), bf16 for
    layers 3-4 (layer-3 values are too small for fp8; layer-4 rows at fp8
    would be under the 256B dma_gather row minimum).
  - Layer-4 aggregation feeds the MLP head block-by-block (short tail).
  - dma_gather indices are int16; AllGather runs as two half-shard collectives
    so gathered row ids stay < 32768.
"""

import os
import math
import numpy as np
import ml_dtypes

import concourse.bass as bass
import concourse.bacc as bacc
import concourse.mybir as mybir
import concourse.tile as tile
from concourse import bass_utils

# problem constants (hardcoded per task contract)
N, E = 50000, 800000
DIN = DH = 512
DE, MH, NCLS = 10, 64, 20
NCORES = 8
NB = 49                   # dst blocks of 128 nodes per core
SHARD = NB * 128          # 6272
NPAD = SHARD * NCORES     # 50176
KC = 4                    # 128-feature chunks of 512

f32 = mybir.dt.float32
f32r = mybir.dt.float32r
f16 = mybir.dt.float16
bf16 = mybir.dt.bfloat16
fp8 = mybir.dt.float8e4
i16 = mybir.dt.int16

# z-path dtype per layer (gather payload + AllGather + self tiles)
ZDTL = {1: fp8, 2: fp8, 3: bf16, 4: bf16}
ZNPL = {l: mybir.dt.np(dt) for l, dt in ZDTL.items()}
DOUT = {1: DH, 2: DH, 3: DH, 4: 128}

# AllGather pieces (block-aligned): fired after dense tiles 24 / 48 of each
# layer.  (A 3-piece split was tried and regressed: collective time is
# floor-dominated at these sizes, so extra pieces cost more than the smaller
# exposed tail saves.)
PBLK = [31, 18]                           # blocks per piece (asymmetric: the
                                          # second piece is fully exposed at
                                          # the phase boundary, so keep it
                                          # small; piece-0 row ids max out at
                                          # 8*3968-1 = 31743, int16-safe)
PROW = [b * 128 for b in PBLK]            # rows per piece: 3968, 2304
PSTART = [0, 3968]
NPIECE = 2

LAST_RESULT = None        # BassKernelResults of the most recent run (for test.py)
_BUILD_CACHE = {}


# ---------------------------------------------------------------- host prep

def _host_prep(x, edge_index):
    src = edge_index[0].astype(np.int64)
    dst = edge_index[1].astype(np.int64)
    deg = np.bincount(dst, minlength=N).astype(np.float32) + 1.0
    dis = (1.0 / np.sqrt(deg)).astype(np.float32)

    sa, da = src, dst
    w = (dis[sa] * dis[da]).astype(np.float32)

    # Segments: src piece membership.  AllGather runs as NPIECE block-aligned
    # piece collectives, so the gathered tensors are zf[j] with rank-major
    # pieces; row ids stay < 32768 (int16-safe) with no extra split.
    r = da // SHARD
    b = (da % SHARD) // 128
    so = sa % SHARD
    s = np.searchsorted(np.array(PSTART[1:]), so, side="right")   # piece idx
    prow = np.array(PROW)[s]
    pstart = np.array(PSTART)[s]
    row = (sa // SHARD) * prow + (so - pstart)     # row in zf[s]
    order = np.lexsort((row, s, b, r))
    row_s, da_s, w_s = row[order], da[order], w[order]
    key = (r * NB + b) * NPIECE + s
    ks = key[order]
    counts = np.bincount(ks, minlength=NCORES * NB * NPIECE)\
        .reshape(NCORES, NB, NPIECE)
    starts = np.zeros(NCORES * NB * NPIECE + 1, np.int64)
    np.cumsum(counts.reshape(-1), out=starts[1:])

    # uniform program structure: per (block, seg) tile count = max over cores
    T_seg = np.ceil(counts / 128.0).astype(np.int64).max(axis=0)  # [NB, NPIECE]
    TT = int(T_seg.sum()) + NB   # +1 self tile per block

    gidx, Sh, Sh8 = [], [], []
    for rr in range(NCORES):
        idx16 = np.zeros(TT * 128, np.int16)
        dcol = np.full(TT * 128, -1, np.int64)
        wcol = np.zeros(TT * 128, np.float32)
        cur = 0
        for bb in range(NB):
            # self tile (tile 0 of each block): S = diag(dis^2) of the
            # block's own nodes; msg row p comes from local z row bb*128+p.
            gnode = rr * SHARD + bb * 128 + np.arange(128)
            ok = gnode < N
            dcol[cur:cur + 128][ok] = np.arange(128)[ok]
            wcol[cur:cur + 128][ok] = (dis[gnode[ok]] ** 2)
            cur += 128
            for ss in range(NPIECE):
                k = (rr * NB + bb) * NPIECE + ss
                n = counts[rr, bb, ss]
                lo, hi = starts[k], starts[k] + n
                idx16[cur:cur + n] = row_s[lo:hi].astype(np.int16)
                dcol[cur:cur + n] = da_s[lo:hi] - rr * SHARD - bb * 128
                wcol[cur:cur + n] = w_s[lo:hi]
                cur += int(T_seg[bb, ss]) * 128   # pads: idx 0, dst -1, w 0
        gidx.append(np.tile(idx16.reshape(-1, 16).T, (8, 1)))       # [128, TT*8]
        # S tiles, host-precomputed: tile e is [128 edges, 128 dst] with
        # S[p, d] = w[e*128+p] iff dst_local[e*128+p] == d.  bf16 copy for
        # layers 3-4, fp8 copy for layers 1-2 (matches the fp8 z payload).
        A = np.zeros((TT * 128, 128), np.float32)
        valid = dcol >= 0
        A[np.nonzero(valid)[0], dcol[valid]] = wcol[valid]
        Ab = A.astype(ml_dtypes.bfloat16).reshape(TT, 128, 128)
        Sh.append(np.ascontiguousarray(Ab.transpose(1, 0, 2).reshape(128, TT * 128)))
        A8 = A.astype(ml_dtypes.float8_e4m3).reshape(TT, 128, 128)
        Sh8.append(np.ascontiguousarray(A8.transpose(1, 0, 2).reshape(128, TT * 128)))

    return {
        "T_seg": T_seg, "TT": TT, "gidx": gidx, "Sh": Sh, "Sh8": Sh8,
    }


def _chunk_w(W):
    """[K, M] -> [128, (K//128)*M] with k-chunk c at cols [c*M, (c+1)*M)."""
    K, M = W.shape
    return np.ascontiguousarray(
        W.reshape(K // 128, 128, M).transpose(1, 0, 2).reshape(128, -1)
    ).astype(np.float16)


# ---------------------------------------------------------------- kernel build

def _build(T_seg_t, TT):
    T_seg = np.asarray(T_seg_t).reshape(NB, NPIECE)
    TBMAX = int(T_seg.sum(axis=1).max()) + 1

    nc = bacc.Bacc("TRN2", target_bir_lowering=False, debug=False,
                   num_devices=NCORES, num_swdge_queues=4)

    dt_in = {}

    def din(name, shape, dt):
        dt_in[name] = nc.dram_tensor(name, shape, dt, kind="ExternalInput")
        return dt_in[name]

    xT = din("xT", [DIN, SHARD], f16)
    Wd = {l: din(f"W{l}", [128, KC * DOUT[l]], f16) for l in (1, 2, 3, 4)}
    Bd = {l: din(f"B{l}", [128, DOUT[l]], f32) for l in (1, 2, 3, 4)}
    M1p = din("M1p", [128, MH], f16)
    M2d = din("M2d", [MH, MH], f16)
    M3d = din("M3d", [MH, NCLS], f16)
    MB1 = din("MB1", [MH, 1], f32)
    MB2 = din("MB2", [MH, 1], f32)
    MB3b = din("MB3b", [128, NCLS], f32)
    ident_c = din("ident_c", [128, 128], f32)
    gidx = din("gidx", [128, TT * 8], i16)
    Sh = din("Sh", [128, TT * 128], bf16)
    Sh8 = din("Sh8", [128, TT * 128], fp8)
    out = nc.dram_tensor("out", [SHARD, NCLS], f32, kind="ExternalOutput")

    # persistent SBUF (h and dense weights in fp16: 10-bit mantissa keeps the
    # dense path's error contribution ~0.05%/elem — bf16 was measured too
    # lossy — while halving the dominant SBUF footprint vs f32r)
    hT = [nc.alloc_sbuf_tensor(f"hT{k}", [128, SHARD], f16).ap() for k in range(KC)]
    W_sb = {p: nc.alloc_sbuf_tensor(f"W_sb{p}", [128, KC * DH], f16).ap()
            for p in (0, 1)}
    W4_sb = nc.alloc_sbuf_tensor("W4_sb", [128, KC * 128], f16).ap()
    ident_sb = nc.alloc_sbuf_tensor("ident_sb", [128, 128], f32).ap()
    Bb_sb = {l: nc.alloc_sbuf_tensor(f"Bb{l}", [128, DOUT[l]], f32).ap()
             for l in (1, 2, 3, 4)}
    M1_sb = nc.alloc_sbuf_tensor("M1_sb", [128, MH], f16).ap()
    M2_sb = nc.alloc_sbuf_tensor("M2_sb", [MH, MH], f16).ap()
    M3_sb = nc.alloc_sbuf_tensor("M3_sb", [MH, NCLS], f16).ap()
    MB1_sb = nc.alloc_sbuf_tensor("MB1_sb", [MH, 1], f32).ap()
    MB2_sb = nc.alloc_sbuf_tensor("MB2_sb", [MH, 1], f32).ap()
    MB3_sb = nc.alloc_sbuf_tensor("MB3_sb", [128, NCLS], f32).ap()

    # DRAM z buffers, one set per layer (race-free pipelining across layers)
    z_loc = {l: nc.dram_tensor(f"zloc{l}", [SHARD, DOUT[l]], ZDTL[l],
                               kind="Internal").ap() for l in (1, 2, 3, 4)}
    zf = {l: tuple(nc.dram_tensor(f"zf{l}_{j}", [NCORES * PROW[j], DOUT[l]],
                                  ZDTL[l], kind="Internal",
                                  addr_space="Shared").ap()
                   for j in range(NPIECE))
          for l in (1, 2, 3, 4)}

    rg = [list(range(NCORES))]

    # column offsets of each block in gidx/Sh
    Tb_of = [1 + int(T_seg[b].sum()) for b in range(NB)]
    col_of = np.concatenate([[0], np.cumsum(Tb_of)]).astype(int)

    with tile.TileContext(nc) as tc:
        with (
            tc.tile_pool(name="meta", bufs=6) as meta,
            tc.tile_pool(name="gp", bufs=4) as gp,
            tc.tile_pool(name="sp", bufs=4) as sp,
            tc.tile_pool(name="zp", bufs=3) as zp,
            tc.tile_pool(name="hp", bufs=3) as hp,
            tc.tile_pool(name="mp", bufs=3) as mp,
            tc.tile_pool(name="ps", bufs=2, space="PSUM") as ps,
            tc.tile_pool(name="psa", bufs=3, space="PSUM") as psa,
            tc.tile_pool(name="pst", bufs=2, space="PSUM") as pst,
        ):
            # ---- constant / weight loads (hT in column chunks so dense L1
            # can start early)
            nc.sync.dma_start(ident_sb, ident_c.ap())
            CCH = SHARD // 4
            for c in range(4):
                for k in range(KC):
                    nc.sync.dma_start(
                        hT[k][:, c * CCH:(c + 1) * CCH],
                        xT[k * 128:(k + 1) * 128, c * CCH:(c + 1) * CCH])
            nc.sync.dma_start(W_sb[1], Wd[1].ap())
            nc.sync.dma_start(W_sb[0], Wd[2].ap())
            nc.sync.dma_start(W4_sb, Wd[4].ap())
            for l in (1, 2, 3, 4):
                nc.sync.dma_start(Bb_sb[l], Bd[l].ap())
            nc.sync.dma_start(M1_sb, M1p.ap())
            nc.sync.dma_start(M2_sb, M2d.ap())
            nc.sync.dma_start(M3_sb, M3d.ap())
            nc.sync.dma_start(MB1_sb, MB1.ap())
            nc.sync.dma_start(MB2_sb, MB2.ap())
            nc.sync.dma_start(MB3_sb, MB3b.ap())

            # SWDGE queue load balancing (greedy by tile count)
            qload = [0, 0, 0, 0]

            def gbuf_tile(l):
                # All layers share one pool slot family (same bytes per slot):
                #   l=1,2: fp8  [128, 2*TBMAX, 512] -> tiles 0..Tb at 512B
                #   l=3:   bf16 [128,   TBMAX, 512] -> tiles at 1024B
                #   l=4:   bf16 [128, 4*TBMAX, 128] -> tiles at 256B
                if l <= 2:
                    return gp.tile([128, 2 * TBMAX, DH], fp8, tag="gbuf",
                                   name="gbuf8")
                if l == 3:
                    return gp.tile([128, TBMAX, DH], bf16, tag="gbuf",
                                   name="gbufh")
                return gp.tile([128, 4 * TBMAX, 128], bf16, tag="gbuf",
                               name="gbuf4")

            def dense_tile(l, t):
                """z_l[t] = hT[:, t] @ W_l -> zdt -> DRAM z_loc."""
                Dout = DOUT[l]
                wsb = W_sb[l % 2] if l < 4 else W4_sb
                zps = ps.tile([128, DH], f32, tag="zps")
                for k in range(KC):
                    nc.tensor.matmul(
                        zps[:, 0:Dout],
                        hT[k][:, t * 128:(t + 1) * 128],
                        wsb[:, k * Dout:(k + 1) * Dout],
                        start=(k == 0), stop=(k == KC - 1),
                    )
                zsb = zp.tile([128, Dout], ZDTL[l], tag=f"zsb_{l}")
                nc.vector.tensor_copy(zsb[:], zps[:, 0:Dout])
                nc.sync.dma_start(z_loc[l][t * 128:(t + 1) * 128, :], zsb[:])

            def fire_ag(l, j):
                nc.gpsimd.collective_compute(
                    "AllGather", mybir.AluOpType.bypass,
                    replica_groups=rg,
                    ins=[z_loc[l][PSTART[j]:PSTART[j] + PROW[j], :]],
                    outs=[zf[l][j]],
                )

            def agg_load(l, b):
                """gbuf alloc + idx/S/self loads + seg-0 gathers for block b."""
                Dout = DOUT[l]
                Tsegs = [int(T_seg[b, j]) for j in range(NPIECE)]
                Tb = 1 + sum(Tsegs)
                col = int(col_of[b])
                gbuf = gbuf_tile(l)
                idx_sb = meta.tile([128, Tb * 8], i16, tag="idx")
                nc.sync.dma_start(idx_sb[:], gidx.ap()[:, col * 8:(col + Tb) * 8])
                if l <= 2:
                    S_sb = sp.tile([128, Tb, 128], fp8, tag="S8")
                    ssrc = Sh8
                else:
                    S_sb = sp.tile([128, Tb, 128], bf16, tag="S")
                    ssrc = Sh
                nc.scalar.dma_start(
                    S_sb[:],
                    ssrc.ap()[:, col * 128:(col + Tb) * 128]
                    .rearrange("p (t d) -> p t d", t=Tb))

                # self tile: contiguous copy of the block's own z rows
                nc.sync.dma_start(gbuf[:, 0:1, :],
                                  z_loc[l][b * 128:(b + 1) * 128, :]
                                  .rearrange("(a p) d -> p a d", a=1))
                ctx = (l, b, Dout, Tsegs, Tb, gbuf, idx_sb, S_sb)
                agg_gather(ctx, 0)
                return ctx

            def agg_gather(ctx, j):
                """Emit the seg-j gathers for block b."""
                l, b, Dout, Tsegs, Tb, gbuf, idx_sb, S_sb = ctx
                off = 1 + sum(Tsegs[:j])
                # SWDGE descriptor ring holds ~1024 descs/queue (ucode
                # constant — it does NOT grow with dynamic_dma_scratch_size;
                # bigger calls hang the device).  Cap at 7 tiles (896 idxs).
                left = Tsegs[j]
                while left > 0:
                    sub = min(7, left)
                    n = sub * 128
                    qn = qload.index(min(qload))
                    qload[qn] += sub
                    nc.gpsimd.dma_gather(
                        gbuf[:, off:off + sub, :],
                        zf[l][j],
                        idx_sb[:, off * 8:(off + sub) * 8],
                        num_idxs=n, num_idxs_reg=n, elem_size=Dout,
                        queue_num=qn,
                    )
                    off += sub
                    left -= sub

            def agg_compute(ctx):
                """S.T @ gbuf accumulation + epilogue for block b."""
                l, b, Dout, Tsegs, Tb, gbuf, idx_sb, S_sb = ctx
                aps = psa.tile([128, DH], f32, tag="aps")
                for e in range(Tb):
                    nc.tensor.matmul(
                        aps[:, 0:Dout], S_sb[:, e, :], gbuf[:, e, :],
                        start=(e == 0), stop=(e == Tb - 1),
                    )

                # epilogue: h = relu(agg + b); transpose back to feature-major
                hsb = hp.tile([128, Dout], f32, tag="hsb")
                nc.vector.tensor_tensor(
                    hsb[:], aps[:, 0:Dout], Bb_sb[l][:, 0:Dout],
                    mybir.AluOpType.add,
                )
                nc.scalar.activation(hsb[:], hsb[:],
                                     mybir.ActivationFunctionType.Relu)
                for k in range(Dout // 128):
                    tps = pst.tile([128, 128], f32, tag="tps")
                    nc.tensor.transpose(tps[:], hsb[:, k * 128:(k + 1) * 128],
                                        ident_sb)
                    nc.vector.tensor_copy(hT[k][:, b * 128:(b + 1) * 128], tps[:])

            def agg_block(l, b):
                ctx = agg_load(l, b)
                for j in range(1, NPIECE):
                    agg_gather(ctx, j)
                agg_compute(ctx)

            def mlp_block(b):
                """out rows of block b from h5 = hT[0][:, b] (128-wide)."""
                sl = slice(b * 128, (b + 1) * 128)
                p5t = ps.tile([128, DH], f32, tag="zps")
                p5 = p5t[0:MH, 0:128]
                nc.tensor.matmul(p5, M1_sb, hT[0][:, sl],
                                 start=True, stop=True)
                h5 = mp.tile([MH, 128], f16, tag="h5")
                nc.scalar.activation(h5[:], p5,
                                     mybir.ActivationFunctionType.Relu,
                                     bias=MB1_sb)
                p6t = psa.tile([128, DH], f32, tag="aps")
                p6 = p6t[0:MH, 0:128]
                nc.tensor.matmul(p6, M2_sb, h5[:], start=True, stop=True)
                h6 = mp.tile([MH, 128], f16, tag="h6")
                nc.scalar.activation(h6[:], p6,
                                     mybir.ActivationFunctionType.Relu,
                                     bias=MB2_sb)
                pot = pst.tile([128, 128], f32, tag="tps")
                po = pot[:, 0:NCLS]
                nc.tensor.matmul(po, h6[:], M3_sb, start=True, stop=True)
                osb = zp.tile([128, NCLS], f32, tag="osb")
                nc.vector.tensor_tensor(osb[:], po, MB3_sb,
                                        mybir.AluOpType.add)
                nc.sync.dma_start(out.ap()[sl, :], osb[:])

            # ---- software pipeline over layers ----
            FIRE_AT = {PBLK[0] - 1: 0}          # piece 0 after dense tile 24
            # layer 1 dense alone (reads x), AG pieces fired asap
            for t in range(NB):
                dense_tile(1, t)
                if t in FIRE_AT:
                    fire_ag(1, FIRE_AT[t])
            fire_ag(1, NPIECE - 1)

            # At each phase start the previous layer's last AllGather piece is
            # still in flight; emit the seg-0 gathers (ready data) of the
            # first K blocks before any seg-1 gather so the gpsimd queue and
            # HBM stay busy through the collective window.
            K = 3
            for l in (2, 3, 4):
                # preload layer-3 dense weights into the now-idle parity
                # buffer (W1/W2/W4 were loaded at startup)
                if l == 3:
                    nc.sync.dma_start(W_sb[1], Wd[3].ap())
                ctxs = [agg_load(l - 1, b) for b in range(K)]
                for b in range(NB):
                    ctx = ctxs[b] if b < K else agg_load(l - 1, b)
                    for j in range(1, NPIECE):
                        agg_gather(ctx, j)
                    agg_compute(ctx)
                    dense_tile(l, b)
                    if b in FIRE_AT:
                        fire_ag(l, FIRE_AT[b])
                fire_ag(l, NPIECE - 1)

            # final aggregation of layer 4 feeding the MLP head per block
            ctxs = [agg_load(4, b) for b in range(K)]
            for b in range(NB):
                ctx = ctxs[b] if b < K else agg_load(4, b)
                for j in range(1, NPIECE):
                    agg_gather(ctx, j)
                agg_compute(ctx)
                mlp_block(b)

    nc.compile()
    return nc


# ---------------------------------------------------------------- entry point

def kernel(x, edge_index, W1, b1, W2, b2, W3, b3, W4, b4,
           M1, mb1, M2, mb2, M3, mb3):
    global LAST_RESULT
    x = np.asarray(x, np.float32)
    edge_index = np.asarray(edge_index)
    meta = _host_prep(x, edge_index)
    key = (tuple(meta["T_seg"].reshape(-1).tolist()), meta["TT"])
    if key not in _BUILD_CACHE:
        _BUILD_CACHE[key] = _build(key[0], key[1])
    nc = _BUILD_CACHE[key]

    W4p = np.zeros((DIN, 128), np.float32)
    W4p[:, :DE] = np.asarray(W4, np.float32)
    b4p = np.zeros(128, np.float32)
    b4p[:DE] = np.asarray(b4, np.float32)
    M1p = np.zeros((128, MH), np.float32)
    M1p[:DE] = np.asarray(M1, np.float32)

    Wch = {1: _chunk_w(np.asarray(W1, np.float32)),
           2: _chunk_w(np.asarray(W2, np.float32)),
           3: _chunk_w(np.asarray(W3, np.float32)),
           4: _chunk_w(W4p)}
    Bb = {1: np.broadcast_to(np.asarray(b1, np.float32), (128, DH)).copy(),
          2: np.broadcast_to(np.asarray(b2, np.float32), (128, DH)).copy(),
          3: np.broadcast_to(np.asarray(b3, np.float32), (128, DH)).copy(),
          4: np.broadcast_to(b4p, (128, 128)).copy()}

    common = {
        **{f"W{l}": Wch[l] for l in (1, 2, 3, 4)},
        **{f"B{l}": Bb[l] for l in (1, 2, 3, 4)},
        "M1p": M1p.astype(np.float16),
        "M2d": np.asarray(M2, np.float16),
        "M3d": np.asarray(M3, np.float16),
        "MB1": np.asarray(mb1, np.float32).reshape(MH, 1),
        "MB2": np.asarray(mb2, np.float32).reshape(MH, 1),
        "MB3b": np.broadcast_to(np.asarray(mb3, np.float32), (128, NCLS)).copy(),
        "ident_c": np.eye(128, dtype=np.float32),
    }

    in_maps = []
    for r in range(NCORES):
        rows = min(SHARD, max(0, N - r * SHARD))
        xp = np.zeros((SHARD, DIN), np.float32)
        xp[:rows] = x[r * SHARD:r * SHARD + rows]
        in_maps.append({
            **common,
            "xT": np.ascontiguousarray(xp.T).astype(np.float16),
            "gidx": meta["gidx"][r],
            "Sh": meta["Sh"][r],
            "Sh8": meta["Sh8"][r],
        })

    LAST_RESULT = bass_utils.run_bass_kernel_spmd(
        nc, in_maps, core_ids=list(range(NCORES)),
    )
    out = np.concatenate([LAST_RESULT.results[r]["out"] for r in range(NCORES)], 0)
    return np.ascontiguousarray(out[:N]).astype(np.float32)


# revision 63
# speedup vs baseline: 1.0197x; 1.0197x over previous
"""Trainium2 Bass kernel for nn_DomainGCN (4-layer GCN + MLP head), 8 NeuronCores.

Strategy (graph/data parallel, per sharding hint):
  - Nodes sharded contiguously: core r owns rows [r*6272, (r+1)*6272) (padded).
  - Reformulation: Agg(h@W) with per-edge weight dis[src]*dis[dst] and
    self-loops as a dedicated per-block self tile (weight dis[j]^2):
      z = h @ W            (dense, f32r matmuls, feature-major h in SBUF)
      z -> zdt -> AllGather (every core gets the full z, node-major, in DRAM)
      agg = S.T @ z[src]   (dma_gather of edge messages + one-hot-weighted
                            matmuls accumulating per 128-dst-node block in PSUM)
      h' = relu(agg + b)   (DVE add + ACT relu), PE-transposed back to
                            feature-major for the next dense matmul.
  - v2: SOFTWARE-PIPELINED emission: agg(l-1, b) and dense(l, b) are emitted
    interleaved per block, so the in-order engines overlap the aggregation of
    layer l-1 with the dense matmuls of layer l, and the AllGather halves of
    layer l fire mid-aggregation (hidden behind gather/matmul work).
    Per-layer z buffers keep this race-free across layers.
  - z payload dtype: fp8e4m3 for layers 1-2 (halves the dominant gather/
    AllGather traffic; verified rel-err ~8.5e-3 vs the 2e-2 gate), bf16 for
    layers 3-4 (layer-3 values are too small for fp8; layer-4 rows at fp8
    would be under the 256B dma_gather row minimum).
  - Layer-4 aggregation feeds the MLP head block-by-block (short tail).
  - dma_gather indices are int16; AllGather runs as two half-shard collectives
    so gathered row ids stay < 32768.
"""

import os
import math
import numpy as np
import ml_dtypes

import concourse.bass as bass
import concourse.bacc as bacc
import concourse.mybir as mybir
import concourse.tile as tile
from concourse import bass_utils

# problem constants (hardcoded per task contract)
N, E = 50000, 800000
DIN = DH = 512
DE, MH, NCLS = 10, 64, 20
NCORES = 8
NB = 49                   # dst blocks of 128 nodes per core
SHARD = NB * 128          # 6272
NPAD = SHARD * NCORES     # 50176
KC = 4                    # 128-feature chunks of 512

f32 = mybir.dt.float32
f32r = mybir.dt.float32r
f16 = mybir.dt.float16
bf16 = mybir.dt.bfloat16
fp8 = mybir.dt.float8e4
i16 = mybir.dt.int16

# z-path dtype per layer (gather payload + AllGather + self tiles)
ZDTL = {1: fp8, 2: fp8, 3: bf16, 4: bf16}
ZNPL = {l: mybir.dt.np(dt) for l, dt in ZDTL.items()}
DOUT = {1: DH, 2: DH, 3: DH, 4: 128}

# AllGather pieces (block-aligned): fired after dense tiles 24 / 48 of each
# layer.  (A 3-piece split was tried and regressed: collective time is
# floor-dominated at these sizes, so extra pieces cost more than the smaller
# exposed tail saves.)
PBLK = [25, 24]                           # blocks per piece (an asymmetric
                                          # [31,18] split was tried and
                                          # regressed +53us: the bigger
                                          # piece-0 collective fires later and
                                          # contends with the phase tail)
PROW = [b * 128 for b in PBLK]            # rows per piece: 3200, 3072
PSTART = [0, 3200]
NPIECE = 2

LAST_RESULT = None        # BassKernelResults of the most recent run (for test.py)
_BUILD_CACHE = {}


# ---------------------------------------------------------------- host prep

def _host_prep(x, edge_index):
    src = edge_index[0].astype(np.int64)
    dst = edge_index[1].astype(np.int64)
    deg = np.bincount(dst, minlength=N).astype(np.float32) + 1.0
    dis = (1.0 / np.sqrt(deg)).astype(np.float32)

    sa, da = src, dst
    w = (dis[sa] * dis[da]).astype(np.float32)

    # Segments: src piece membership.  AllGather runs as NPIECE block-aligned
    # piece collectives, so the gathered tensors are zf[j] with rank-major
    # pieces; row ids stay < 32768 (int16-safe) with no extra split.
    r = da // SHARD
    b = (da % SHARD) // 128
    so = sa % SHARD
    s = np.searchsorted(np.array(PSTART[1:]), so, side="right")   # piece idx
    prow = np.array(PROW)[s]
    pstart = np.array(PSTART)[s]
    row = (sa // SHARD) * prow + (so - pstart)     # row in zf[s]
    order = np.lexsort((row, s, b, r))
    row_s, da_s, w_s = row[order], da[order], w[order]
    key = (r * NB + b) * NPIECE + s
    ks = key[order]
    counts = np.bincount(ks, minlength=NCORES * NB * NPIECE)\
        .reshape(NCORES, NB, NPIECE)
    starts = np.zeros(NCORES * NB * NPIECE + 1, np.int64)
    np.cumsum(counts.reshape(-1), out=starts[1:])

    # uniform program structure: per (block, seg) tile count = max over cores
    T_seg = np.ceil(counts / 128.0).astype(np.int64).max(axis=0)  # [NB, NPIECE]
    TT = int(T_seg.sum()) + NB   # +1 self tile per block

    gidx, Sh, Sh8 = [], [], []
    for rr in range(NCORES):
        idx16 = np.zeros(TT * 128, np.int16)
        dcol = np.full(TT * 128, -1, np.int64)
        wcol = np.zeros(TT * 128, np.float32)
        cur = 0
        for bb in range(NB):
            # self tile (tile 0 of each block): S = diag(dis^2) of the
            # block's own nodes; msg row p comes from local z row bb*128+p.
            gnode = rr * SHARD + bb * 128 + np.arange(128)
            ok = gnode < N
            dcol[cur:cur + 128][ok] = np.arange(128)[ok]
            wcol[cur:cur + 128][ok] = (dis[gnode[ok]] ** 2)
            cur += 128
            for ss in range(NPIECE):
                k = (rr * NB + bb) * NPIECE + ss
                n = counts[rr, bb, ss]
                lo, hi = starts[k], starts[k] + n
                idx16[cur:cur + n] = row_s[lo:hi].astype(np.int16)
                dcol[cur:cur + n] = da_s[lo:hi] - rr * SHARD - bb * 128
                wcol[cur:cur + n] = w_s[lo:hi]
                cur += int(T_seg[bb, ss]) * 128   # pads: idx 0, dst -1, w 0
        gidx.append(np.tile(idx16.reshape(-1, 16).T, (8, 1)))       # [128, TT*8]
        # S tiles, host-precomputed: tile e is [128 edges, 128 dst] with
        # S[p, d] = w[e*128+p] iff dst_local[e*128+p] == d.  bf16 copy for
        # layers 3-4, fp8 copy for layers 1-2 (matches the fp8 z payload).
        A = np.zeros((TT * 128, 128), np.float32)
        valid = dcol >= 0
        A[np.nonzero(valid)[0], dcol[valid]] = wcol[valid]
        Ab = A.astype(ml_dtypes.bfloat16).reshape(TT, 128, 128)
        Sh.append(np.ascontiguousarray(Ab.transpose(1, 0, 2).reshape(128, TT * 128)))
        A8 = A.astype(ml_dtypes.float8_e4m3).reshape(TT, 128, 128)
        Sh8.append(np.ascontiguousarray(A8.transpose(1, 0, 2).reshape(128, TT * 128)))

    return {
        "T_seg": T_seg, "TT": TT, "gidx": gidx, "Sh": Sh, "Sh8": Sh8,
    }


def _chunk_w(W):
    """[K, M] -> [128, (K//128)*M] with k-chunk c at cols [c*M, (c+1)*M)."""
    K, M = W.shape
    return np.ascontiguousarray(
        W.reshape(K // 128, 128, M).transpose(1, 0, 2).reshape(128, -1)
    ).astype(np.float16)


# ---------------------------------------------------------------- kernel build

def _build(T_seg_t, TT):
    T_seg = np.asarray(T_seg_t).reshape(NB, NPIECE)
    TBMAX = int(T_seg.sum(axis=1).max()) + 1

    nc = bacc.Bacc("TRN2", target_bir_lowering=False, debug=False,
                   num_devices=NCORES, num_swdge_queues=4)

    dt_in = {}

    def din(name, shape, dt):
        dt_in[name] = nc.dram_tensor(name, shape, dt, kind="ExternalInput")
        return dt_in[name]

    xT = din("xT", [DIN, SHARD], f16)
    Wd = {l: din(f"W{l}", [128, KC * DOUT[l]], f16) for l in (1, 2, 3, 4)}
    Bd = {l: din(f"B{l}", [128, DOUT[l]], f32) for l in (1, 2, 3, 4)}
    M1p = din("M1p", [128, MH], f16)
    M2d = din("M2d", [MH, MH], f16)
    M3d = din("M3d", [MH, NCLS], f16)
    MB1 = din("MB1", [MH, 1], f32)
    MB2 = din("MB2", [MH, 1], f32)
    MB3b = din("MB3b", [128, NCLS], f32)
    ident_c = din("ident_c", [128, 128], f32)
    gidx = din("gidx", [128, TT * 8], i16)
    Sh = din("Sh", [128, TT * 128], bf16)
    Sh8 = din("Sh8", [128, TT * 128], fp8)
    out = nc.dram_tensor("out", [SHARD, NCLS], f32, kind="ExternalOutput")

    # persistent SBUF (h and dense weights in fp16: 10-bit mantissa keeps the
    # dense path's error contribution ~0.05%/elem — bf16 was measured too
    # lossy — while halving the dominant SBUF footprint vs f32r)
    hT = [nc.alloc_sbuf_tensor(f"hT{k}", [128, SHARD], f16).ap() for k in range(KC)]
    W_sb = {p: nc.alloc_sbuf_tensor(f"W_sb{p}", [128, KC * DH], f16).ap()
            for p in (0, 1)}
    W4_sb = nc.alloc_sbuf_tensor("W4_sb", [128, KC * 128], f16).ap()
    ident_sb = nc.alloc_sbuf_tensor("ident_sb", [128, 128], f32).ap()
    Bb_sb = {l: nc.alloc_sbuf_tensor(f"Bb{l}", [128, DOUT[l]], f32).ap()
             for l in (1, 2, 3, 4)}
    M1_sb = nc.alloc_sbuf_tensor("M1_sb", [128, MH], f16).ap()
    M2_sb = nc.alloc_sbuf_tensor("M2_sb", [MH, MH], f16).ap()
    M3_sb = nc.alloc_sbuf_tensor("M3_sb", [MH, NCLS], f16).ap()
    MB1_sb = nc.alloc_sbuf_tensor("MB1_sb", [MH, 1], f32).ap()
    MB2_sb = nc.alloc_sbuf_tensor("MB2_sb", [MH, 1], f32).ap()
    MB3_sb = nc.alloc_sbuf_tensor("MB3_sb", [128, NCLS], f32).ap()

    # DRAM z buffers, one set per layer (race-free pipelining across layers)
    z_loc = {l: nc.dram_tensor(f"zloc{l}", [SHARD, DOUT[l]], ZDTL[l],
                               kind="Internal").ap() for l in (1, 2, 3, 4)}
    zf = {l: tuple(nc.dram_tensor(f"zf{l}_{j}", [NCORES * PROW[j], DOUT[l]],
                                  ZDTL[l], kind="Internal",
                                  addr_space="Shared").ap()
                   for j in range(NPIECE))
          for l in (1, 2, 3, 4)}

    rg = [list(range(NCORES))]

    # column offsets of each block in gidx/Sh
    Tb_of = [1 + int(T_seg[b].sum()) for b in range(NB)]
    col_of = np.concatenate([[0], np.cumsum(Tb_of)]).astype(int)

    with tile.TileContext(nc) as tc:
        with (
            tc.tile_pool(name="meta", bufs=6) as meta,
            tc.tile_pool(name="gp", bufs=3) as gp,
            tc.tile_pool(name="sp", bufs=4) as sp,
            tc.tile_pool(name="zp", bufs=3) as zp,
            tc.tile_pool(name="hp", bufs=3) as hp,
            tc.tile_pool(name="mp", bufs=3) as mp,
            tc.tile_pool(name="ps", bufs=2, space="PSUM") as ps,
            tc.tile_pool(name="psa", bufs=3, space="PSUM") as psa,
            tc.tile_pool(name="pst", bufs=2, space="PSUM") as pst,
        ):
            # ---- constant / weight loads (hT in column chunks so dense L1
            # can start early)
            # startup loads split across both HWDGE rings (scalar is otherwise
            # idle until the first agg phase) so dense L1 and the first
            # AllGather fire sooner
            nc.scalar.dma_start(W_sb[1], Wd[1].ap())
            nc.sync.dma_start(ident_sb, ident_c.ap())
            CCH = SHARD // 4
            for c in range(4):
                for k in range(KC):
                    eng = nc.sync if k % 2 == 0 else nc.scalar
                    eng.dma_start(
                        hT[k][:, c * CCH:(c + 1) * CCH],
                        xT[k * 128:(k + 1) * 128, c * CCH:(c + 1) * CCH])
            nc.sync.dma_start(W_sb[0], Wd[2].ap())
            nc.sync.dma_start(W4_sb, Wd[4].ap())
            for l in (1, 2, 3, 4):
                nc.sync.dma_start(Bb_sb[l], Bd[l].ap())
            nc.sync.dma_start(M1_sb, M1p.ap())
            nc.sync.dma_start(M2_sb, M2d.ap())
            nc.sync.dma_start(M3_sb, M3d.ap())
            nc.sync.dma_start(MB1_sb, MB1.ap())
            nc.sync.dma_start(MB2_sb, MB2.ap())
            nc.sync.dma_start(MB3_sb, MB3b.ap())

            # SWDGE queue load balancing (greedy by tile count)
            qload = [0, 0, 0, 0]

            def gbuf_tile(l):
                # All layers share one pool slot family (same bytes per slot):
                #   l=1,2: fp8  [128, 2*TBMAX, 512] -> tiles 0..Tb at 512B
                #   l=3:   bf16 [128,   TBMAX, 512] -> tiles at 1024B
                #   l=4:   bf16 [128, 4*TBMAX, 128] -> tiles at 256B
                if l <= 2:
                    return gp.tile([128, 2 * TBMAX, DH], fp8, tag="gbuf",
                                   name="gbuf8")
                if l == 3:
                    return gp.tile([128, TBMAX, DH], bf16, tag="gbuf",
                                   name="gbufh")
                return gp.tile([128, 4 * TBMAX, 128], bf16, tag="gbuf",
                               name="gbuf4")

            def dense_tile(l, t):
                """z_l[t] = hT[:, t] @ W_l -> zdt -> DRAM z_loc."""
                Dout = DOUT[l]
                wsb = W_sb[l % 2] if l < 4 else W4_sb
                zps = ps.tile([128, DH], f32, tag="zps")
                for k in range(KC):
                    nc.tensor.matmul(
                        zps[:, 0:Dout],
                        hT[k][:, t * 128:(t + 1) * 128],
                        wsb[:, k * Dout:(k + 1) * Dout],
                        start=(k == 0), stop=(k == KC - 1),
                    )
                zsb = zp.tile([128, Dout], ZDTL[l], tag=f"zsb_{l}")
                nc.vector.tensor_copy(zsb[:], zps[:, 0:Dout])
                nc.sync.dma_start(z_loc[l][t * 128:(t + 1) * 128, :], zsb[:])

            def fire_ag(l, j):
                nc.gpsimd.collective_compute(
                    "AllGather", mybir.AluOpType.bypass,
                    replica_groups=rg,
                    ins=[z_loc[l][PSTART[j]:PSTART[j] + PROW[j], :]],
                    outs=[zf[l][j]],
                )

            def agg_load(l, b):
                """gbuf alloc + idx/S/self loads + seg-0 gathers for block b."""
                Dout = DOUT[l]
                Tsegs = [int(T_seg[b, j]) for j in range(NPIECE)]
                Tb = 1 + sum(Tsegs)
                col = int(col_of[b])
                gbuf = gbuf_tile(l)
                idx_sb = meta.tile([128, Tb * 8], i16, tag="idx")
                nc.sync.dma_start(idx_sb[:], gidx.ap()[:, col * 8:(col + Tb) * 8])
                if l <= 2:
                    S_sb = sp.tile([128, Tb, 128], fp8, tag="S8")
                    ssrc = Sh8
                else:
                    S_sb = sp.tile([128, Tb, 128], bf16, tag="S")
                    ssrc = Sh
                nc.scalar.dma_start(
                    S_sb[:],
                    ssrc.ap()[:, col * 128:(col + Tb) * 128]
                    .rearrange("p (t d) -> p t d", t=Tb))

                # self tile: contiguous copy of the block's own z rows
                nc.sync.dma_start(gbuf[:, 0:1, :],
                                  z_loc[l][b * 128:(b + 1) * 128, :]
                                  .rearrange("(a p) d -> p a d", a=1))
                ctx = (l, b, Dout, Tsegs, Tb, gbuf, idx_sb, S_sb)
                agg_gather(ctx, 0)
                return ctx

            def agg_gather(ctx, j):
                """Emit the seg-j gathers for block b."""
                l, b, Dout, Tsegs, Tb, gbuf, idx_sb, S_sb = ctx
                off = 1 + sum(Tsegs[:j])
                # SWDGE descriptor ring holds ~1024 descs/queue (ucode
                # constant — it does NOT grow with dynamic_dma_scratch_size;
                # bigger calls hang the device).  Cap at 7 tiles (896 idxs).
                left = Tsegs[j]
                while left > 0:
                    sub = min(7, left)
                    n = sub * 128
                    qn = qload.index(min(qload))
                    qload[qn] += sub
                    nc.gpsimd.dma_gather(
                        gbuf[:, off:off + sub, :],
                        zf[l][j],
                        idx_sb[:, off * 8:(off + sub) * 8],
                        num_idxs=n, num_idxs_reg=n, elem_size=Dout,
                        queue_num=qn,
                    )
                    off += sub
                    left -= sub

            def agg_compute(ctx):
                """S.T @ gbuf accumulation + epilogue for block b."""
                l, b, Dout, Tsegs, Tb, gbuf, idx_sb, S_sb = ctx
                aps = psa.tile([128, DH], f32, tag="aps")
                for e in range(Tb):
                    nc.tensor.matmul(
                        aps[:, 0:Dout], S_sb[:, e, :], gbuf[:, e, :],
                        start=(e == 0), stop=(e == Tb - 1),
                    )

                # epilogue: h = relu(agg + b); transpose back to feature-major
                hsb = hp.tile([128, Dout], f32, tag="hsb")
                nc.vector.tensor_tensor(
                    hsb[:], aps[:, 0:Dout], Bb_sb[l][:, 0:Dout],
                    mybir.AluOpType.add,
                )
                nc.scalar.activation(hsb[:], hsb[:],
                                     mybir.ActivationFunctionType.Relu)
                for k in range(Dout // 128):
                    tps = pst.tile([128, 128], f32, tag="tps")
                    nc.tensor.transpose(tps[:], hsb[:, k * 128:(k + 1) * 128],
                                        ident_sb)
                    nc.vector.tensor_copy(hT[k][:, b * 128:(b + 1) * 128], tps[:])

            def agg_block(l, b):
                ctx = agg_load(l, b)
                for j in range(1, NPIECE):
                    agg_gather(ctx, j)
                agg_compute(ctx)

            def mlp_block(b):
                """out rows of block b from h5 = hT[0][:, b] (128-wide)."""
                sl = slice(b * 128, (b + 1) * 128)
                p5t = ps.tile([128, DH], f32, tag="zps")
                p5 = p5t[0:MH, 0:128]
                nc.tensor.matmul(p5, M1_sb, hT[0][:, sl],
                                 start=True, stop=True)
                h5 = mp.tile([MH, 128], f16, tag="h5")
                nc.scalar.activation(h5[:], p5,
                                     mybir.ActivationFunctionType.Relu,
                                     bias=MB1_sb)
                p6t = psa.tile([128, DH], f32, tag="aps")
                p6 = p6t[0:MH, 0:128]
                nc.tensor.matmul(p6, M2_sb, h5[:], start=True, stop=True)
                h6 = mp.tile([MH, 128], f16, tag="h6")
                nc.scalar.activation(h6[:], p6,
                                     mybir.ActivationFunctionType.Relu,
                                     bias=MB2_sb)
                pot = pst.tile([128, 128], f32, tag="tps")
                po = pot[:, 0:NCLS]
                nc.tensor.matmul(po, h6[:], M3_sb, start=True, stop=True)
                osb = zp.tile([128, NCLS], f32, tag="osb")
                nc.vector.tensor_tensor(osb[:], po, MB3_sb,
                                        mybir.AluOpType.add)
                nc.sync.dma_start(out.ap()[sl, :], osb[:])

            # ---- software pipeline over layers ----
            FIRE_AT = {PBLK[0] - 1: 0}          # piece 0 after dense tile 24
            # layer 1 dense alone (reads x), AG pieces fired asap
            for t in range(NB):
                dense_tile(1, t)
                if t in FIRE_AT:
                    fire_ag(1, FIRE_AT[t])
            fire_ag(1, NPIECE - 1)

            # At each phase start the previous layer's last AllGather piece is
            # still in flight; emit the seg-0 gathers (ready data) of the
            # first K blocks before any seg-1 gather so the gpsimd queue and
            # HBM stay busy through the collective window.
            K = 3
            for l in (2, 3, 4):
                # preload layer-3 dense weights into the now-idle parity
                # buffer (W1/W2/W4 were loaded at startup)
                if l == 3:
                    nc.sync.dma_start(W_sb[1], Wd[3].ap())
                ctxs = [agg_load(l - 1, b) for b in range(K)]
                for b in range(NB):
                    ctx = ctxs[b] if b < K else agg_load(l - 1, b)
                    for j in range(1, NPIECE):
                        agg_gather(ctx, j)
                    agg_compute(ctx)
                    dense_tile(l, b)
                    if b in FIRE_AT:
                        fire_ag(l, FIRE_AT[b])
                fire_ag(l, NPIECE - 1)

            # final aggregation of layer 4 feeding the MLP head per block
            ctxs = [agg_load(4, b) for b in range(K)]
            for b in range(NB):
                ctx = ctxs[b] if b < K else agg_load(4, b)
                for j in range(1, NPIECE):
                    agg_gather(ctx, j)
                agg_compute(ctx)
                mlp_block(b)

    nc.compile()
    return nc


# ---------------------------------------------------------------- entry point

def kernel(x, edge_index, W1, b1, W2, b2, W3, b3, W4, b4,
           M1, mb1, M2, mb2, M3, mb3):
    global LAST_RESULT
    x = np.asarray(x, np.float32)
    edge_index = np.asarray(edge_index)
    meta = _host_prep(x, edge_index)
    key = (tuple(meta["T_seg"].reshape(-1).tolist()), meta["TT"])
    if key not in _BUILD_CACHE:
        _BUILD_CACHE[key] = _build(key[0], key[1])
    nc = _BUILD_CACHE[key]

    W4p = np.zeros((DIN, 128), np.float32)
    W4p[:, :DE] = np.asarray(W4, np.float32)
    b4p = np.zeros(128, np.float32)
    b4p[:DE] = np.asarray(b4, np.float32)
    M1p = np.zeros((128, MH), np.float32)
    M1p[:DE] = np.asarray(M1, np.float32)

    Wch = {1: _chunk_w(np.asarray(W1, np.float32)),
           2: _chunk_w(np.asarray(W2, np.float32)),
           3: _chunk_w(np.asarray(W3, np.float32)),
           4: _chunk_w(W4p)}
    Bb = {1: np.broadcast_to(np.asarray(b1, np.float32), (128, DH)).copy(),
          2: np.broadcast_to(np.asarray(b2, np.float32), (128, DH)).copy(),
          3: np.broadcast_to(np.asarray(b3, np.float32), (128, DH)).copy(),
          4: np.broadcast_to(b4p, (128, 128)).copy()}

    common = {
        **{f"W{l}": Wch[l] for l in (1, 2, 3, 4)},
        **{f"B{l}": Bb[l] for l in (1, 2, 3, 4)},
        "M1p": M1p.astype(np.float16),
        "M2d": np.asarray(M2, np.float16),
        "M3d": np.asarray(M3, np.float16),
        "MB1": np.asarray(mb1, np.float32).reshape(MH, 1),
        "MB2": np.asarray(mb2, np.float32).reshape(MH, 1),
        "MB3b": np.broadcast_to(np.asarray(mb3, np.float32), (128, NCLS)).copy(),
        "ident_c": np.eye(128, dtype=np.float32),
    }

    in_maps = []
    for r in range(NCORES):
        rows = min(SHARD, max(0, N - r * SHARD))
        xp = np.zeros((SHARD, DIN), np.float32)
        xp[:rows] = x[r * SHARD:r * SHARD + rows]
        in_maps.append({
            **common,
            "xT": np.ascontiguousarray(xp.T).astype(np.float16),
            "gidx": meta["gidx"][r],
            "Sh": meta["Sh"][r],
            "Sh8": meta["Sh8"][r],
        })

    LAST_RESULT = bass_utils.run_bass_kernel_spmd(
        nc, in_maps, core_ids=list(range(NCORES)),
    )
    out = np.concatenate([LAST_RESULT.results[r]["out"] for r in range(NCORES)], 0)
    return np.ascontiguousarray(out[:N]).astype(np.float32)
